# revision 27
# baseline (speedup 1.0000x reference)
"""Multi-head causal attention with RoPE on 8 Trainium2 NeuronCores.

Sharding: data-parallel over batch (B=2) x tensor-parallel over heads
(16 heads -> 4 groups of 4). Core c handles batch c//4, heads
[(c%4)*4, (c%4)*4+4). Each core computes a partial y = attn_out @ W_o
for its head group; the host sums the 4 partials per batch (the "W_o
all-reduce").

Device kernel (per core, matmuls bf16, fp32 PSUM accumulation):
  - x^T pre-transposed on host; Q^T/K^T projections in T layout
    (dims on partitions, seq on free) accumulated over 8 E-chunks.
  - RoPE pair shuffle folded into a host permutation of W_q/W_k
    columns; on device one DVE stream_shuffle + cos/sin multiply-adds.
  - V projected directly in NATURAL layout (t on partitions) as
    out = x^T_block.T @ W_v chunks -> no PE transposes; a ones column
    per head accumulates softmax denominators during PV.
  - attention: head-pair outer, 4 q-passes of width 512. Per t-block:
    scores^T[t, q] = K^T.T @ Qz (zero-padded K=128), ONE merged exp
    over both heads ([128, 2, 512] PSUM tile) on ACT, causal mask mul
    on the diagonal blocks ([128, 2, 128] with a doubled mask tile),
    PV software-pipelined one iteration behind.
  - epilogue per pass: PSUM-freeing copies immediately; reciprocal/
    broadcast/normalize chains deferred into the next pass.
  - V-natural blocks are emitted just-in-time inside head-pair 0's
    passes; the output projection (onrm^T.T @ W_o) for q-pass p is
    injected into head-pair 1's pass p+1 so PE never idles at the end.
"""

import os
import sys
from contextlib import ExitStack

import numpy as np

for _p in ("/opt/trn_rl_repo",):
    if os.path.isdir(_p) and _p not in sys.path:
        sys.path.insert(0, _p)

import ml_dtypes  # noqa: E402

BF16 = ml_dtypes.bfloat16

B, S, E = 2, 2048, 1024
H, DH = 16, 64
NCORES = 8
HPC = H // 4          # 4 heads per core
DC = HPC * DH         # 256 head dims per core
ATTN_SCALE = 1.0 / 32.0  # 1/sqrt(E)
ROPE_BASE = 10000.0
P = 128
NSB = S // P          # 16 sequence blocks
NEC = E // P          # 8 E chunks
MB = DC // P          # 2 partition blocks of head dims
PW = 512              # attention q-pass width
NPASS = S // PW       # 4

_PROG = None


def _perm64():
    """perm[j] = original head-dim index stored at permuted position j.

    Quadrant q of the permuted layout holds RoPE pairs i in
    [16q, 16q+16): even elements (2i) at slots 0-15, odd (2i+1) at
    slots 16-31. The rotation partner is then always +-16 partitions
    away within one 32-partition quadrant (stream_shuffle range).
    """
    j = np.arange(64)
    qd, r = j // 32, j % 32
    i = 16 * qd + (r % 16)
    return 2 * i + (r >= 16)


def _cos_sin_tiles():
    pl = np.arange(P) % 64
    qd, r = pl // 32, pl % 32
    i = 16 * qd + (r % 16)
    inv = ROPE_BASE ** (-(2.0 * i) / DH)
    ang = np.arange(S)[None, :] * inv[:, None]          # (128, S)
    sgn = np.where(r < 16, -1.0, 1.0)[:, None]
    return ang, sgn


def _build_program():
    import concourse.bacc as bacc
    import concourse.tile as tile
    from concourse import mybir

    f32 = mybir.dt.float32
    bf16 = mybir.dt.bfloat16
    AF = mybir.ActivationFunctionType

    nc = bacc.Bacc("TRN2", target_bir_lowering=False, debug=False)
    # weights come pre-rearranged from host into on-chip layout
    # ([P, chunk, cols] flattened) so every DMA is contiguous per
    # partition -- the (c p) m -> p c m gather DMA was a 6us startup
    # stall.
    xbt = nc.dram_tensor("xbt", [E, S], bf16, kind="ExternalInput").ap()
    wq = nc.dram_tensor("wq", [P, NEC * DC], bf16, kind="ExternalInput").ap()
    wk = nc.dram_tensor("wk", [P, NEC * DC], bf16, kind="ExternalInput").ap()
    wv = nc.dram_tensor("wv", [P, NEC * DC], bf16, kind="ExternalInput").ap()
    wo = nc.dram_tensor("wo", [P, MB * E], bf16, kind="ExternalInput").ap()
    cosr = nc.dram_tensor("cosr", [P, S], bf16, kind="ExternalInput").ap()
    sinr = nc.dram_tensor("sinr", [P, S], bf16, kind="ExternalInput").ap()
    cmask = nc.dram_tensor("cmask", [P, P], bf16, kind="ExternalInput").ap()
    y = nc.dram_tensor("y", [S, E], f32, kind="ExternalOutput").ap()

    with ExitStack() as ctx:
        tc = ctx.enter_context(tile.TileContext(nc))
        consts = ctx.enter_context(tc.tile_pool(name="consts", bufs=1))
        persist = ctx.enter_context(tc.tile_pool(name="persist", bufs=1))

        qcT = persist.tile([P, MB, S], bf16, tag="qcT")
        kcT = persist.tile([P, MB, S], bf16, tag="kcT")
        # qz: RoPE'd Q^T zero-padded per head parity: slice
        # [:, mb, par, :] has head (2*mb+par)'s 64 rows live, other 64
        # rows zero, so scores can use the full K=128 (the HAM clock
        # gate never grants full clock to K=64 streams).
        qz = persist.tile([P, MB, 2, S], bf16, tag="qz")
        kT = persist.tile([P, MB, S], bf16, tag="kT")
        vn = persist.tile([P, NSB, HPC, DH + 1], bf16, tag="vn")
        onrm = persist.tile([P, MB, S], bf16, tag="onrm")

        xp = ctx.enter_context(tc.tile_pool(name="xp", bufs=1))
        xT = xp.tile([P, NEC, S], bf16, tag="xT")

        wk_t = consts.tile([P, NEC, DC], bf16, tag="wk")
        wq_t = consts.tile([P, NEC, DC], bf16, tag="wq")
        wv_t = consts.tile([P, NEC, DC], bf16, tag="wv")
        wo_t = consts.tile([P, MB, E], bf16, tag="wo")
        cos_t = consts.tile([P, S], bf16, tag="cos")
        sin_t = consts.tile([P, S], bf16, tag="sin")
        msk2 = consts.tile([P, 2, P], bf16, tag="msk2")

        # ---- input DMAs, ordered for earliest first matmul ----
        # K-proj is DMA-paced at the start: wk in 2 pieces so the first
        # accumulate only waits chunks 0-3; x^T chunks interleaved
        # across both rings; wq between x chunks (needed ~12us in).
        wkf = wk_t[:].rearrange("p c m -> p (c m)")
        wqf = wq_t[:].rearrange("p c m -> p (c m)")
        # scalar ring
        nc.scalar.dma_start(wkf[:, 0:4 * DC], wk[:, 0:4 * DC])
        nc.scalar.dma_start(wkf[:, 4 * DC:8 * DC], wk[:, 4 * DC:8 * DC])
        nc.scalar.dma_start(xT[:, 1, :], xbt[P:2 * P, :])
        nc.scalar.dma_start(xT[:, 3, :], xbt[3 * P:4 * P, :])
        nc.scalar.dma_start(sin_t[:], sinr)
        nc.scalar.dma_start(xT[:, 5, :], xbt[5 * P:6 * P, :])
        nc.scalar.dma_start(xT[:, 7, :], xbt[7 * P:8 * P, :])
        nc.scalar.dma_start(wv_t[:].rearrange("p c m -> p (c m)"), wv)
        nc.scalar.dma_start(msk2[:, 0, :], cmask)
        nc.scalar.dma_start(msk2[:, 1, :], cmask)
        # sync ring
        nc.sync.dma_start(xT[:, 0, 0:S // 2], xbt[0:P, 0:S // 2])
        nc.sync.dma_start(xT[:, 0, S // 2:S], xbt[0:P, S // 2:S])
        nc.sync.dma_start(xT[:, 2, :], xbt[2 * P:3 * P, :])
        nc.sync.dma_start(wq_t[:, 0:4, :].rearrange("p c m -> p (c m)"),
                          wq[:, 0:4 * DC])
        nc.sync.dma_start(xT[:, 4, :], xbt[4 * P:5 * P, :])
        nc.sync.dma_start(cos_t[:], cosr)
        nc.sync.dma_start(xT[:, 6, :], xbt[6 * P:7 * P, :])
        nc.sync.dma_start(wq_t[:, 4:8, :].rearrange("p c m -> p (c m)"),
                          wq[:, 4 * DC:8 * DC])
        nc.sync.dma_start(wo_t[:].rearrange("p c m -> p (c m)"), wo)

        shuf_mask = list(range(16, 32)) + list(range(16))
        nc.gpsimd.memset(qz[0:DH, :, 1, :], 0.0)
        nc.gpsimd.memset(qz[DH:P, :, 0, :], 0.0)
        nc.vector.memset(vn[:, :, :, DH:DH + 1], 1.0)

        def rope_k(mb, eng, pool, c0=0, c1=S, nm=""):
            # eng=gpsimd offloads the combine off DVE; all APs are
            # full-width base-0 (gpsimd mishandles partition offsets)
            sh = pool.tile([P, S], bf16, tag="shuf", name=f"shk{mb}{nm}")
            nc.vector.stream_shuffle(sh[:, c0:c1], kcT[:, mb, c0:c1],
                                     shuf_mask)
            eng.tensor_mul(sh[:, c0:c1], sh[:, c0:c1], sin_t[:, c0:c1])
            eng.tensor_mul(kT[:, mb, c0:c1], kcT[:, mb, c0:c1],
                           cos_t[:, c0:c1])
            eng.tensor_add(kT[:, mb, c0:c1], kT[:, mb, c0:c1], sh[:, c0:c1])

        def rope_q(mb, pool, c0=0, c1=S, nm=""):
            sh = pool.tile([P, S], bf16, tag="shuf", name=f"shq{mb}{nm}")
            nc.vector.stream_shuffle(sh[:, c0:c1], qcT[:, mb, c0:c1],
                                     shuf_mask)
            nc.vector.tensor_mul(sh[:, c0:c1], sh[:, c0:c1], sin_t[:, c0:c1])
            for par in range(2):
                o0 = par * DH
                nc.vector.tensor_mul(
                    qz[o0:o0 + DH, mb, par, c0:c1],
                    qcT[o0:o0 + DH, mb, c0:c1],
                    cos_t[o0:o0 + DH, c0:c1],
                )
                nc.vector.tensor_add(
                    qz[o0:o0 + DH, mb, par, c0:c1],
                    qz[o0:o0 + DH, mb, par, c0:c1],
                    sh[o0:o0 + DH, c0:c1],
                )

        # B-phase SBUF pools allocated up front: if they lived in the
        # phase-B scope they would reuse (alias) phase A's scratch
        # addresses and the first exp would wait on the whole rope tail.
        ptp = ctx.enter_context(tc.tile_pool(name="ptp", bufs=4))
        dn = ctx.enter_context(tc.tile_pool(name="dn", bufs=2))
        yo = ctx.enter_context(tc.tile_pool(name="yo", bufs=3))
        rtmp = ctx.enter_context(tc.tile_pool(name="rtmp", bufs=2))

        def vnat(sb, pool, w, tag):
            # V in natural layout: [128 t, 256 d] accumulated over
            # E-chunks, then one strided copy into vn.
            vp = pool.tile([P, w], f32, tag=tag, name=f"vp{sb}")
            for ec in range(NEC):
                nc.tensor.matmul(
                    vp[:, 0:DC],
                    lhsT=xT[:, ec, sb * P:(sb + 1) * P],
                    rhs=wv_t[:, ec, :],
                    start=(ec == 0),
                    stop=(ec == NEC - 1),
                )
            nc.vector.tensor_copy(
                vn[:, sb, :, 0:DH],
                vp[:, 0:DC].rearrange("p (h d) -> p h d", h=HPC),
            )

        # ---- Phase A: Q/K projections + RoPE (V is deferred) ----
        # All 4 projections (K1, Q1, K0, Q0) interleave at the E-chunk
        # level: each x^T chunk DMA feeds 4 accumulating matmuls, so
        # the PE tracks the DMA stream instead of idling between
        # chunks four times over. S-half 0 of all four uses the full
        # 8 PSUM banks; half 1 reuses the ring.
        with ExitStack() as actx:
            pr_ps = actx.enter_context(
                tc.tile_pool(name="pr_ps", bufs=4, space="PSUM")
            )
            projs = [
                (wk_t, kcT, 1, None),
                (wq_t, qcT, 1, None),
                (wk_t, kcT, 0,
                 lambda c0, nm: rope_k(0, nc.vector, rtmp, c0, c0 + 512, nm)),
                (wq_t, qcT, 0,
                 lambda c0, nm: rope_q(0, rtmp, c0, c0 + 512, nm)),
            ]

            def proj_round(half):
                tiles = []
                for j, (wt, dst, mb, rope) in enumerate(projs):
                    tiles.append(pr_ps.tile([P, S // 2], f32, tag="proj",
                                            name=f"pj{mb}{dst is qcT}_{half}"))
                for i in range(NEC):
                    for j, (wt, dst, mb, rope) in enumerate(projs):
                        for qt in range(2):
                            c0 = half * 1024 + qt * 512
                            nc.tensor.matmul(
                                tiles[j][:, qt * 512:(qt + 1) * 512],
                                lhsT=wt[:, i, mb * P:(mb + 1) * P],
                                rhs=xT[:, i, c0:c0 + 512],
                                start=(i == 0),
                                stop=(i == NEC - 1),
                            )
                for j, (wt, dst, mb, rope) in enumerate(projs):
                    sl = slice(half * 1024, (half + 1) * 1024)
                    if j % 2 == 0:
                        nc.vector.tensor_copy(dst[:, mb, sl], tiles[j][:])
                    else:
                        nc.scalar.copy(dst[:, mb, sl], tiles[j][:])
                    if rope is not None:
                        # quarter-granularity: attention pass 0 only
                        # needs cols 0-512 of kT/qz, so it starts two
                        # quarter-chains after the half-0 copies
                        rope(half * 1024, f"h{half}")

            proj_round(0)
            for sb_i in range(4):
                vnat(sb_i, pr_ps, S // 2, "proj")
            proj_round(1)
            rope_k(0, nc.vector, rtmp, 512, 1024, "q1")
            rope_q(0, rtmp, 512, 1024, "q1")
            rope_k(0, nc.vector, rtmp, 1536, 2048, "q3")
            rope_q(0, rtmp, 1536, 2048, "q3")

        # ---- Phase B: attention + V-natural + output projection ----
        with ExitStack() as bctx:
            sc_ps = bctx.enter_context(
                tc.tile_pool(name="sc_ps", bufs=2, space="PSUM")
            )
            ac_ps = bctx.enter_context(
                tc.tile_pool(name="ac_ps", bufs=1, space="PSUM")
            )
            # late V-natural blocks + out-proj accumulators (disjoint
            # in time, shared ring)
            sp_ps = bctx.enter_context(
                tc.tile_pool(name="sp_ps", bufs=2, space="PSUM")
            )

            def epilogue_copies(h, q0, accs):
                # stage out^T + the denominator row out of PSUM (plain
                # DVE copies -- these two free the accumulator bank).
                acb = dn.tile([DH, PW], f32, tag=f"acb{h % 2}",
                              name=f"acb{h}_{q0}")
                nc.vector.tensor_copy(acb[:], accs[h][0:DH, :])
                den0 = dn.tile([1, PW], f32, tag=f"den{h % 2}",
                               name=f"den{h}_{q0}")
                nc.vector.tensor_copy(den0[:], accs[h][DH:DH + 1, :])
                return h, q0, acb, den0

            def epilogue_norm(h, q0, acb, den0):
                # 2-pass approximate reciprocal, partition broadcast on
                # GpSimd (reads partition 0 of a base-0 tile), one mul.
                rden = dn.tile([1, PW], f32, tag=f"rden{h % 2}",
                               name=f"rden{h}_{q0}")
                # ~18 bits is plenty against a 2e-2 output tolerance
                nc.vector.reciprocal_approx_fast(rden[:], den0[:])
                rdb = dn.tile([DH, PW], f32, tag=f"rdb{h % 2}",
                              name=f"rdb{h}_{q0}")
                nc.gpsimd.partition_broadcast(rdb[:], rden[:])
                mb, off = h // 2, (h % 2) * DH
                nc.vector.tensor_mul(
                    onrm[off:off + DH, mb, q0:q0 + PW], acb[:], rdb[:]
                )

            def yproj_unit(sb, half):
                yp = sp_ps.tile([P, PW], f32, tag="sp", name=f"yp{sb}_{half}")
                for mb2 in range(MB):
                    nc.tensor.matmul(
                        yp[:],
                        lhsT=onrm[:, mb2, sb * P:(sb + 1) * P],
                        rhs=wo_t[:, mb2, half * PW:(half + 1) * PW],
                        start=(mb2 == 0),
                        stop=(mb2 == MB - 1),
                    )
                ys = yo.tile([P, PW], f32, tag="ys", name=f"ys{sb}_{half}")
                if (sb + half) % 2 == 0:
                    nc.vector.tensor_copy(ys[:], yp[:])
                else:
                    nc.scalar.copy(ys[:], yp[:])
                # y DMAs all on the (idle) sync ring -- a scalar-ring DMA
                # costs ~600ns on the queue that paces the exps.
                nc.sync.dma_start(
                    y[sb * P:(sb + 1) * P, half * PW:(half + 1) * PW], ys[:]
                )

            deferred_norms = []
            yq = []
            for hp in range(2):
                heads = (2 * hp, 2 * hp + 1)
                mb = hp
                for p in range(NPASS):
                    q0 = p * PW
                    nti = 4 * (p + 1)
                    accs = {
                        h: ac_ps.tile([DH + 1, PW], f32, tag=f"acc{h % 2}",
                                      name=f"acc{h}_{p}")
                        for h in heads
                    }

                    def issue_pv(h, ti, pt, l0):
                        nc.tensor.matmul(
                            accs[h][:, l0:PW],
                            lhsT=vn[:, ti, h, :],
                            rhs=pt[:, h % 2, l0:PW],
                            start=(ti == 0),
                            stop=(ti == nti - 1),
                        )

                    pending = []
                    for ti in range(nti):
                        if hp == 0 and p >= 1 and ti < 4:
                            vnat(4 * p + ti, sp_ps, PW, "sp")
                        if hp == 0 and p == 2 and ti == 0:
                            # mb=1 RoPE: emitted mid-way through head
                            # pair 0 so its DVE work doesn't queue ahead
                            # of the first vn copies / mask muls; done
                            # long before head-pair 1 needs it.
                            rope_k(1, nc.gpsimd, dn)
                            rope_q(1, dn)
                        if ti == 2 and deferred_norms:
                            # previous pass's normalize chains, deferred
                            # so the PSUM-release semaphores fire first
                            for st in deferred_norms:
                                epilogue_norm(*st)
                            deferred_norms = []
                        if ti >= 3 and yq:
                            yq.pop(0)()
                            if yq:
                                yq.pop(0)()
                        t0 = ti * P
                        l0 = max(t0 - q0, 0)
                        sc = sc_ps.tile([P, 2, PW], f32, tag="sc",
                                        name=f"sc{hp}_{p}_{ti}")
                        for h in heads:
                            nc.tensor.matmul(
                                sc[:, h % 2, l0:PW],
                                lhsT=kT[:, mb, t0:t0 + P],
                                rhs=qz[:, mb, h % 2, q0 + l0:q0 + PW],
                            )
                        pt = ptp.tile([P, 2, PW], bf16, tag="pt",
                                      name=f"pt{hp}_{p}_{ti}")
                        nc.scalar.activation(
                            pt[:, :, l0:PW],
                            sc[:, :, l0:PW],
                            AF.Exp,
                            scale=ATTN_SCALE,
                        )
                        if t0 >= q0:
                            nc.vector.tensor_mul(
                                pt[:, :, l0:l0 + P],
                                pt[:, :, l0:l0 + P],
                                msk2[:],
                            )
                        for args in pending:
                            issue_pv(*args)
                        pending = [(h, ti, pt, l0) for h in heads]
                    for args in pending:
                        issue_pv(*args)
                    while yq:
                        yq.pop(0)()

                    last = hp == 1 and p == NPASS - 1
                    if last:
                        # tail fast path: skip the acb staging (PSUM
                        # release isn't urgent), normalize straight from
                        # the accumulators so yproj starts ~3us sooner
                        rdbs = {}
                        for h in heads:
                            den0 = dn.tile([1, PW], f32, tag=f"den{h % 2}",
                                           name=f"den{h}_{q0}")
                            nc.vector.tensor_copy(
                                den0[:], accs[h][DH:DH + 1, :]
                            )
                            rden = dn.tile([1, PW], f32, tag=f"rden{h % 2}",
                                           name=f"rden{h}_{q0}")
                            nc.vector.reciprocal_approx_fast(rden[:], den0[:])
                            rdb = dn.tile([DH, PW], f32, tag=f"rdb{h % 2}",
                                          name=f"rdb{h}_{q0}")
                            nc.gpsimd.partition_broadcast(rdb[:], rden[:])
                            rdbs[h] = rdb
                        for h in heads:
                            mb2, off = h // 2, (h % 2) * DH
                            nc.vector.tensor_mul(
                                onrm[off:off + DH, mb2, q0:q0 + PW],
                                accs[h][0:DH, :],
                                rdbs[h][:],
                            )
                    else:
                        deferred_norms = [
                            epilogue_copies(h, q0, accs) for h in heads
                        ]
                    if hp == 1:
                        # out-projection for q-pass p, injected into the
                        # next pass (p = last: emitted right below)
                        units = [
                            (4 * p + sb_i, half)
                            for sb_i in range(4)
                            for half in range(2)
                        ]
                        if last:
                            for sb_i, half in units:
                                yproj_unit(sb_i, half)
                        else:
                            yq = [
                                (lambda s=s, hf=hf: yproj_unit(s, hf))
                                for s, hf in units
                            ]

    nc.compile()
    return nc


def get_program():
    global _PROG
    if _PROG is None:
        _PROG = _build_program()
    return _PROG


def make_in_maps(x, W_q, W_k, W_v, W_o):
    perm = _perm64()
    idx_local = (np.arange(DC) // 64) * 64 + perm[np.arange(DC) % 64]
    ang, sgn = _cos_sin_tiles()
    cos_np = np.cos(ang).astype(BF16)
    sin_np = (sgn * np.sin(ang)).astype(BF16)
    # scores tile is (t, q): keep t <= q -> upper triangular incl. diagonal
    cmask_np = np.triu(np.ones((P, P))).astype(BF16)
    def chunked(w):
        # [E, cols] -> on-chip layout [P, NEC * cols]: chunk c of rows
        # lands at [:, c, :] so device DMAs are contiguous.
        cols = w.shape[1]
        return np.ascontiguousarray(
            w.reshape(NEC, P, cols).transpose(1, 0, 2).reshape(P, NEC * cols)
            .astype(BF16)
        )

    def chunked2(w):
        cols = w.shape[1]
        return np.ascontiguousarray(
            w.reshape(MB, P, cols).transpose(1, 0, 2).reshape(P, MB * cols)
            .astype(BF16)
        )

    in_maps = []
    for c in range(NCORES):
        b, hg = c // 4, c % 4
        base = hg * DC
        in_maps.append(
            dict(
                xbt=np.ascontiguousarray(x[b].T.astype(BF16)),
                wq=chunked(W_q[:, base + idx_local]),
                wk=chunked(W_k[:, base + idx_local]),
                wv=chunked(W_v[:, base:base + DC]),
                wo=chunked2(W_o[base:base + DC, :]),
                cosr=cos_np,
                sinr=sin_np,
                cmask=cmask_np,
            )
        )
    return in_maps


def kernel(x, W_q, W_k, W_v, W_o, _trace=False, _trace_cores=None):
    from concourse.bass_utils import run_bass_kernel_spmd

    x = np.asarray(x, dtype=np.float32)
    W_q = np.asarray(W_q, dtype=np.float32)
    W_k = np.asarray(W_k, dtype=np.float32)
    W_v = np.asarray(W_v, dtype=np.float32)
    W_o = np.asarray(W_o, dtype=np.float32)

    nc = get_program()
    in_maps = make_in_maps(x, W_q, W_k, W_v, W_o)
    res = run_bass_kernel_spmd(
        nc,
        in_maps,
        list(range(NCORES)),
        trace=_trace,
        trace_cores=_trace_cores,
    )
    y = np.zeros((B, S, E), np.float32)
    for c in range(NCORES):
        y[c // 4] += res.results[c]["y"]
    if _trace:
        return y, res
    return y


# revision 28
# speedup vs baseline: 1.1852x; 1.1852x over previous
"""Multi-head causal attention with RoPE on 8 Trainium2 NeuronCores.

Sharding: data-parallel over batch (B=2) x tensor-parallel over heads
(16 heads -> 4 groups of 4). Core c handles batch c//4, heads
[(c%4)*4, (c%4)*4+4). Each core computes a partial y = attn_out @ W_o
for its head group; the host sums the 4 partials per batch (the "W_o
all-reduce").

Device kernel (per core, matmuls bf16, fp32 PSUM accumulation):
  - x^T pre-transposed on host; Q^T/K^T projections in T layout
    (dims on partitions, seq on free) accumulated over 8 E-chunks.
  - RoPE pair shuffle folded into a host permutation of W_q/W_k
    columns; on device one DVE stream_shuffle + cos/sin multiply-adds.
  - V projected directly in NATURAL layout (t on partitions) as
    out = x^T_block.T @ W_v chunks -> no PE transposes; a ones column
    per head accumulates softmax denominators during PV.
  - attention: head-pair outer, 4 q-passes of width 512. Per t-block:
    scores^T[t, q] = K^T.T @ Qz (zero-padded K=128), ONE merged exp
    over both heads ([128, 2, 512] PSUM tile) on ACT, causal mask mul
    on the diagonal blocks ([128, 2, 128] with a doubled mask tile),
    PV software-pipelined one iteration behind.
  - epilogue per pass: PSUM-freeing copies immediately; reciprocal/
    broadcast/normalize chains deferred into the next pass.
  - V-natural blocks are emitted just-in-time inside head-pair 0's
    passes; the output projection (onrm^T.T @ W_o) for q-pass p is
    injected into head-pair 1's pass p+1 so PE never idles at the end.
"""

import os
import sys
from contextlib import ExitStack

import numpy as np

for _p in ("/opt/trn_rl_repo",):
    if os.path.isdir(_p) and _p not in sys.path:
        sys.path.insert(0, _p)

import ml_dtypes  # noqa: E402

BF16 = ml_dtypes.bfloat16

B, S, E = 2, 2048, 1024
H, DH = 16, 64
NCORES = 8
HPC = H // 4          # 4 heads per core
DC = HPC * DH         # 256 head dims per core
ATTN_SCALE = 1.0 / 32.0  # 1/sqrt(E)
ROPE_BASE = 10000.0
P = 128
NSB = S // P          # 16 sequence blocks
NEC = E // P          # 8 E chunks
MB = DC // P          # 2 partition blocks of head dims
PW = 512              # attention q-pass width
NPASS = S // PW       # 4

_PROG = None


def _perm64():
    """perm[j] = original head-dim index stored at permuted position j.

    Quadrant q of the permuted layout holds RoPE pairs i in
    [16q, 16q+16): even elements (2i) at slots 0-15, odd (2i+1) at
    slots 16-31. The rotation partner is then always +-16 partitions
    away within one 32-partition quadrant (stream_shuffle range).
    """
    j = np.arange(64)
    qd, r = j // 32, j % 32
    i = 16 * qd + (r % 16)
    return 2 * i + (r >= 16)


def _cos_sin_tiles():
    pl = np.arange(P) % 64
    qd, r = pl // 32, pl % 32
    i = 16 * qd + (r % 16)
    inv = ROPE_BASE ** (-(2.0 * i) / DH)
    ang = np.arange(S)[None, :] * inv[:, None]          # (128, S)
    sgn = np.where(r < 16, -1.0, 1.0)[:, None]
    return ang, sgn


def _build_program():
    import concourse.bacc as bacc
    import concourse.tile as tile
    from concourse import mybir

    f32 = mybir.dt.float32
    bf16 = mybir.dt.bfloat16
    AF = mybir.ActivationFunctionType

    nc = bacc.Bacc("TRN2", target_bir_lowering=False, debug=False)
    # weights come pre-rearranged from host into on-chip layout
    # ([P, chunk, cols] flattened) so every DMA is contiguous per
    # partition -- the (c p) m -> p c m gather DMA was a 6us startup
    # stall.
    xbt = nc.dram_tensor("xbt", [E, S], bf16, kind="ExternalInput").ap()
    wq = nc.dram_tensor("wq", [P, NEC * DC], bf16, kind="ExternalInput").ap()
    wk = nc.dram_tensor("wk", [P, NEC * DC], bf16, kind="ExternalInput").ap()
    wv = nc.dram_tensor("wv", [P, NEC * DC], bf16, kind="ExternalInput").ap()
    wo = nc.dram_tensor("wo", [P, MB * E], bf16, kind="ExternalInput").ap()
    cosr = nc.dram_tensor("cosr", [P, S], bf16, kind="ExternalInput").ap()
    sinr = nc.dram_tensor("sinr", [P, S], bf16, kind="ExternalInput").ap()
    cmask = nc.dram_tensor("cmask", [P, P], bf16, kind="ExternalInput").ap()
    y = nc.dram_tensor("y", [S, E], f32, kind="ExternalOutput").ap()

    with ExitStack() as ctx:
        tc = ctx.enter_context(tile.TileContext(nc))
        consts = ctx.enter_context(tc.tile_pool(name="consts", bufs=1))
        persist = ctx.enter_context(tc.tile_pool(name="persist", bufs=1))

        qcT = persist.tile([P, MB, S], bf16, tag="qcT")
        kcT = persist.tile([P, MB, S], bf16, tag="kcT")
        # qz: RoPE'd Q^T zero-padded per head parity: slice
        # [:, mb, par, :] has head (2*mb+par)'s 64 rows live, other 64
        # rows zero, so scores can use the full K=128 (the HAM clock
        # gate never grants full clock to K=64 streams).
        qz = persist.tile([P, MB, 2, S], bf16, tag="qz")
        kT = persist.tile([P, MB, S], bf16, tag="kT")
        vn = persist.tile([P, NSB, HPC, DH + 1], bf16, tag="vn")
        onrm = persist.tile([P, MB, S], bf16, tag="onrm")

        xp = ctx.enter_context(tc.tile_pool(name="xp", bufs=1))
        xT = xp.tile([P, NEC, S], bf16, tag="xT")

        wk_t = consts.tile([P, NEC, DC], bf16, tag="wk")
        wq_t = consts.tile([P, NEC, DC], bf16, tag="wq")
        wv_t = consts.tile([P, NEC, DC], bf16, tag="wv")
        wo_t = consts.tile([P, MB, E], bf16, tag="wo")
        cos_t = consts.tile([P, S], bf16, tag="cos")
        sin_t = consts.tile([P, S], bf16, tag="sin")
        msk2 = consts.tile([P, 2, P], bf16, tag="msk2")

        # ---- input DMAs, ordered for earliest first matmul ----
        # K-proj is DMA-paced at the start: wk in 2 pieces so the first
        # accumulate only waits chunks 0-3; x^T chunks interleaved
        # across both rings; wq between x chunks (needed ~12us in).
        wkf = wk_t[:].rearrange("p c m -> p (c m)")
        wqf = wq_t[:].rearrange("p c m -> p (c m)")
        # scalar ring
        nc.scalar.dma_start(wkf[:, 0:4 * DC], wk[:, 0:4 * DC])
        nc.scalar.dma_start(wkf[:, 4 * DC:8 * DC], wk[:, 4 * DC:8 * DC])
        nc.scalar.dma_start(xT[:, 1, :], xbt[P:2 * P, :])
        nc.scalar.dma_start(xT[:, 3, :], xbt[3 * P:4 * P, :])
        nc.scalar.dma_start(sin_t[:], sinr)
        nc.scalar.dma_start(xT[:, 5, :], xbt[5 * P:6 * P, :])
        nc.scalar.dma_start(xT[:, 7, :], xbt[7 * P:8 * P, :])
        nc.scalar.dma_start(wv_t[:].rearrange("p c m -> p (c m)"), wv)
        nc.scalar.dma_start(msk2[:, 0, :], cmask)
        nc.scalar.dma_start(msk2[:, 1, :], cmask)
        # sync ring
        nc.sync.dma_start(xT[:, 0, 0:S // 2], xbt[0:P, 0:S // 2])
        nc.sync.dma_start(xT[:, 0, S // 2:S], xbt[0:P, S // 2:S])
        nc.sync.dma_start(xT[:, 2, :], xbt[2 * P:3 * P, :])
        nc.sync.dma_start(wq_t[:, 0:4, :].rearrange("p c m -> p (c m)"),
                          wq[:, 0:4 * DC])
        nc.sync.dma_start(xT[:, 4, :], xbt[4 * P:5 * P, :])
        nc.sync.dma_start(cos_t[:], cosr)
        nc.sync.dma_start(xT[:, 6, :], xbt[6 * P:7 * P, :])
        nc.sync.dma_start(wq_t[:, 4:8, :].rearrange("p c m -> p (c m)"),
                          wq[:, 4 * DC:8 * DC])
        nc.sync.dma_start(wo_t[:].rearrange("p c m -> p (c m)"), wo)

        shuf_mask = list(range(16, 32)) + list(range(16))
        nc.gpsimd.memset(qz[0:DH, :, 1, :], 0.0)
        nc.gpsimd.memset(qz[DH:P, :, 0, :], 0.0)
        nc.vector.memset(vn[:, :, :, DH:DH + 1], 1.0)

        def rope_k(mb, eng, pool, c0=0, c1=S, nm=""):
            # eng=gpsimd offloads the combine off DVE; all APs are
            # full-width base-0 (gpsimd mishandles partition offsets)
            sh = pool.tile([P, S], bf16, tag="shuf", name=f"shk{mb}{nm}")
            nc.vector.stream_shuffle(sh[:, c0:c1], kcT[:, mb, c0:c1],
                                     shuf_mask)
            eng.tensor_mul(sh[:, c0:c1], sh[:, c0:c1], sin_t[:, c0:c1])
            eng.tensor_mul(kT[:, mb, c0:c1], kcT[:, mb, c0:c1],
                           cos_t[:, c0:c1])
            eng.tensor_add(kT[:, mb, c0:c1], kT[:, mb, c0:c1], sh[:, c0:c1])

        def rope_q(mb, pool, c0=0, c1=S, nm=""):
            sh = pool.tile([P, S], bf16, tag="shuf", name=f"shq{mb}{nm}")
            nc.vector.stream_shuffle(sh[:, c0:c1], qcT[:, mb, c0:c1],
                                     shuf_mask)
            nc.vector.tensor_mul(sh[:, c0:c1], sh[:, c0:c1], sin_t[:, c0:c1])
            for par in range(2):
                o0 = par * DH
                nc.vector.tensor_mul(
                    qz[o0:o0 + DH, mb, par, c0:c1],
                    qcT[o0:o0 + DH, mb, c0:c1],
                    cos_t[o0:o0 + DH, c0:c1],
                )
                nc.vector.tensor_add(
                    qz[o0:o0 + DH, mb, par, c0:c1],
                    qz[o0:o0 + DH, mb, par, c0:c1],
                    sh[o0:o0 + DH, c0:c1],
                )

        # B-phase SBUF pools allocated up front: if they lived in the
        # phase-B scope they would reuse (alias) phase A's scratch
        # addresses and the first exp would wait on the whole rope tail.
        ptp = ctx.enter_context(tc.tile_pool(name="ptp", bufs=4))
        dn = ctx.enter_context(tc.tile_pool(name="dn", bufs=2))
        yo = ctx.enter_context(tc.tile_pool(name="yo", bufs=3))
        rtmp = ctx.enter_context(tc.tile_pool(name="rtmp", bufs=2))

        def vnat(sb, pool, w, tag):
            # V in natural layout: [128 t, 256 d] accumulated over
            # E-chunks, then one strided copy into vn.
            vp = pool.tile([P, w], f32, tag=tag, name=f"vp{sb}")
            for ec in range(NEC):
                nc.tensor.matmul(
                    vp[:, 0:DC],
                    lhsT=xT[:, ec, sb * P:(sb + 1) * P],
                    rhs=wv_t[:, ec, :],
                    start=(ec == 0),
                    stop=(ec == NEC - 1),
                )
            nc.vector.tensor_copy(
                vn[:, sb, :, 0:DH],
                vp[:, 0:DC].rearrange("p (h d) -> p h d", h=HPC),
            )

        # ---- Phase A: Q/K projections + RoPE (V is deferred) ----
        # All 4 projections (K1, Q1, K0, Q0) interleave at the E-chunk
        # level: each x^T chunk DMA feeds 4 accumulating matmuls, so
        # the PE tracks the DMA stream instead of idling between
        # chunks four times over. S-half 0 of all four uses the full
        # 8 PSUM banks; half 1 reuses the ring.
        with ExitStack() as actx:
            pr_ps = actx.enter_context(
                tc.tile_pool(name="pr_ps", bufs=4, space="PSUM")
            )
            projs = [
                (wk_t, kcT, 1, None),
                (wq_t, qcT, 1, None),
                (wk_t, kcT, 0,
                 lambda c0, nm: rope_k(0, nc.vector, rtmp, c0, c0 + 512, nm)),
                (wq_t, qcT, 0,
                 lambda c0, nm: rope_q(0, rtmp, c0, c0 + 512, nm)),
            ]

            def proj_round(half):
                tiles = []
                for j, (wt, dst, mb, rope) in enumerate(projs):
                    tiles.append(pr_ps.tile([P, S // 2], f32, tag="proj",
                                            name=f"pj{mb}{dst is qcT}_{half}"))
                for i in range(NEC):
                    for j, (wt, dst, mb, rope) in enumerate(projs):
                        for qt in range(2):
                            c0 = half * 1024 + qt * 512
                            nc.tensor.matmul(
                                tiles[j][:, qt * 512:(qt + 1) * 512],
                                lhsT=wt[:, i, mb * P:(mb + 1) * P],
                                rhs=xT[:, i, c0:c0 + 512],
                                start=(i == 0),
                                stop=(i == NEC - 1),
                            )
                for j, (wt, dst, mb, rope) in enumerate(projs):
                    sl = slice(half * 1024, (half + 1) * 1024)
                    if j % 2 == 0:
                        nc.vector.tensor_copy(dst[:, mb, sl], tiles[j][:])
                    else:
                        nc.scalar.copy(dst[:, mb, sl], tiles[j][:])
                    if rope is not None:
                        # quarter-granularity: attention pass 0 only
                        # needs cols 0-512 of kT/qz, so it starts two
                        # quarter-chains after the half-0 copies
                        rope(half * 1024, f"h{half}")

            proj_round(0)
            for sb_i in range(4):
                vnat(sb_i, pr_ps, S // 2, "proj")
            # cols 512-1024 are half-0 data: rope them BEFORE the
            # half-1 round so attention pass 1 isn't stuck behind the
            # half-1 copies on the DVE queue
            rope_k(0, nc.vector, rtmp, 512, 1024, "q1")
            rope_q(0, rtmp, 512, 1024, "q1")
            proj_round(1)
            rope_k(0, nc.vector, rtmp, 1536, 2048, "q3")
            rope_q(0, rtmp, 1536, 2048, "q3")

        # ---- Phase B: attention + V-natural + output projection ----
        with ExitStack() as bctx:
            sc_ps = bctx.enter_context(
                tc.tile_pool(name="sc_ps", bufs=2, space="PSUM")
            )
            ac_ps = bctx.enter_context(
                tc.tile_pool(name="ac_ps", bufs=1, space="PSUM")
            )
            # late V-natural blocks + out-proj accumulators (disjoint
            # in time, shared ring)
            sp_ps = bctx.enter_context(
                tc.tile_pool(name="sp_ps", bufs=2, space="PSUM")
            )

            def epilogue_copies(h, q0, accs):
                # stage out^T + the denominator row out of PSUM (plain
                # DVE copies -- these two free the accumulator bank).
                acb = dn.tile([DH, PW], f32, tag=f"acb{h % 2}",
                              name=f"acb{h}_{q0}")
                nc.vector.tensor_copy(acb[:], accs[h][0:DH, :])
                den0 = dn.tile([1, PW], f32, tag=f"den{h % 2}",
                               name=f"den{h}_{q0}")
                nc.vector.tensor_copy(den0[:], accs[h][DH:DH + 1, :])
                return h, q0, acb, den0

            def epilogue_norm(h, q0, acb, den0):
                # 2-pass approximate reciprocal, partition broadcast on
                # GpSimd (reads partition 0 of a base-0 tile), one mul.
                rden = dn.tile([1, PW], f32, tag=f"rden{h % 2}",
                               name=f"rden{h}_{q0}")
                # ~18 bits is plenty against a 2e-2 output tolerance
                nc.vector.reciprocal_approx_fast(rden[:], den0[:])
                rdb = dn.tile([DH, PW], f32, tag=f"rdb{h % 2}",
                              name=f"rdb{h}_{q0}")
                nc.gpsimd.partition_broadcast(rdb[:], rden[:])
                mb, off = h // 2, (h % 2) * DH
                nc.vector.tensor_mul(
                    onrm[off:off + DH, mb, q0:q0 + PW], acb[:], rdb[:]
                )

            def yproj_unit(sb, half):
                yp = sp_ps.tile([P, PW], f32, tag="sp", name=f"yp{sb}_{half}")
                for mb2 in range(MB):
                    nc.tensor.matmul(
                        yp[:],
                        lhsT=onrm[:, mb2, sb * P:(sb + 1) * P],
                        rhs=wo_t[:, mb2, half * PW:(half + 1) * PW],
                        start=(mb2 == 0),
                        stop=(mb2 == MB - 1),
                    )
                ys = yo.tile([P, PW], f32, tag="ys", name=f"ys{sb}_{half}")
                if (sb + half) % 2 == 0:
                    nc.vector.tensor_copy(ys[:], yp[:])
                else:
                    nc.scalar.copy(ys[:], yp[:])
                # y DMAs all on the (idle) sync ring -- a scalar-ring DMA
                # costs ~600ns on the queue that paces the exps.
                nc.sync.dma_start(
                    y[sb * P:(sb + 1) * P, half * PW:(half + 1) * PW], ys[:]
                )

            deferred_norms = []
            yq = []
            for hp in range(2):
                heads = (2 * hp, 2 * hp + 1)
                mb = hp
                for p in range(NPASS):
                    q0 = p * PW
                    nti = 4 * (p + 1)
                    accs = {
                        h: ac_ps.tile([DH + 1, PW], f32, tag=f"acc{h % 2}",
                                      name=f"acc{h}_{p}")
                        for h in heads
                    }

                    def issue_pv(h, ti, pt, l0):
                        nc.tensor.matmul(
                            accs[h][:, l0:PW],
                            lhsT=vn[:, ti, h, :],
                            rhs=pt[:, h % 2, l0:PW],
                            start=(ti == 0),
                            stop=(ti == nti - 1),
                        )

                    pending = []
                    for ti in range(nti):
                        if hp == 0 and p >= 1 and ti < 4:
                            vnat(4 * p + ti, sp_ps, PW, "sp")
                        if hp == 0 and p == 2 and ti == 0:
                            # mb=1 RoPE: emitted mid-way through head
                            # pair 0 so its DVE work doesn't queue ahead
                            # of the first vn copies / mask muls; done
                            # long before head-pair 1 needs it.
                            rope_k(1, nc.gpsimd, dn)
                            rope_q(1, dn)
                        if ti == 2 and deferred_norms:
                            # previous pass's normalize chains, deferred
                            # so the PSUM-release semaphores fire first
                            for st in deferred_norms:
                                epilogue_norm(*st)
                            deferred_norms = []
                        if ti >= 3 and yq:
                            yq.pop(0)()
                            if yq:
                                yq.pop(0)()
                        t0 = ti * P
                        l0 = max(t0 - q0, 0)
                        sc = sc_ps.tile([P, 2, PW], f32, tag="sc",
                                        name=f"sc{hp}_{p}_{ti}")
                        for h in heads:
                            nc.tensor.matmul(
                                sc[:, h % 2, l0:PW],
                                lhsT=kT[:, mb, t0:t0 + P],
                                rhs=qz[:, mb, h % 2, q0 + l0:q0 + PW],
                            )
                        pt = ptp.tile([P, 2, PW], bf16, tag="pt",
                                      name=f"pt{hp}_{p}_{ti}")
                        nc.scalar.activation(
                            pt[:, :, l0:PW],
                            sc[:, :, l0:PW],
                            AF.Exp,
                            scale=ATTN_SCALE,
                        )
                        if t0 >= q0:
                            nc.vector.tensor_mul(
                                pt[:, :, l0:l0 + P],
                                pt[:, :, l0:l0 + P],
                                msk2[:],
                            )
                        for args in pending:
                            issue_pv(*args)
                        pending = [(h, ti, pt, l0) for h in heads]
                    for args in pending:
                        issue_pv(*args)
                    while yq:
                        yq.pop(0)()

                    last = hp == 1 and p == NPASS - 1
                    if last:
                        # tail fast path: skip the acb staging (PSUM
                        # release isn't urgent), normalize straight from
                        # the accumulators so yproj starts ~3us sooner
                        rdbs = {}
                        for h in heads:
                            den0 = dn.tile([1, PW], f32, tag=f"den{h % 2}",
                                           name=f"den{h}_{q0}")
                            nc.vector.tensor_copy(
                                den0[:], accs[h][DH:DH + 1, :]
                            )
                            rden = dn.tile([1, PW], f32, tag=f"rden{h % 2}",
                                           name=f"rden{h}_{q0}")
                            nc.vector.reciprocal_approx_fast(rden[:], den0[:])
                            rdb = dn.tile([DH, PW], f32, tag=f"rdb{h % 2}",
                                          name=f"rdb{h}_{q0}")
                            nc.gpsimd.partition_broadcast(rdb[:], rden[:])
                            rdbs[h] = rdb
                        for h in heads:
                            mb2, off = h // 2, (h % 2) * DH
                            nc.vector.tensor_mul(
                                onrm[off:off + DH, mb2, q0:q0 + PW],
                                accs[h][0:DH, :],
                                rdbs[h][:],
                            )
                    else:
                        deferred_norms = [
                            epilogue_copies(h, q0, accs) for h in heads
                        ]
                    if hp == 1:
                        # out-projection for q-pass p, injected into the
                        # next pass (p = last: emitted right below)
                        units = [
                            (4 * p + sb_i, half)
                            for sb_i in range(4)
                            for half in range(2)
                        ]
                        if last:
                            for sb_i, half in units:
                                yproj_unit(sb_i, half)
                        else:
                            yq = [
                                (lambda s=s, hf=hf: yproj_unit(s, hf))
                                for s, hf in units
                            ]

    nc.compile()
    return nc


def get_program():
    global _PROG
    if _PROG is None:
        _PROG = _build_program()
    return _PROG


def make_in_maps(x, W_q, W_k, W_v, W_o):
    perm = _perm64()
    idx_local = (np.arange(DC) // 64) * 64 + perm[np.arange(DC) % 64]
    ang, sgn = _cos_sin_tiles()
    cos_np = np.cos(ang).astype(BF16)
    sin_np = (sgn * np.sin(ang)).astype(BF16)
    # scores tile is (t, q): keep t <= q -> upper triangular incl. diagonal
    cmask_np = np.triu(np.ones((P, P))).astype(BF16)
    def chunked(w):
        # [E, cols] -> on-chip layout [P, NEC * cols]: chunk c of rows
        # lands at [:, c, :] so device DMAs are contiguous.
        cols = w.shape[1]
        return np.ascontiguousarray(
            w.reshape(NEC, P, cols).transpose(1, 0, 2).reshape(P, NEC * cols)
            .astype(BF16)
        )

    def chunked2(w):
        cols = w.shape[1]
        return np.ascontiguousarray(
            w.reshape(MB, P, cols).transpose(1, 0, 2).reshape(P, MB * cols)
            .astype(BF16)
        )

    in_maps = []
    for c in range(NCORES):
        b, hg = c // 4, c % 4
        base = hg * DC
        in_maps.append(
            dict(
                xbt=np.ascontiguousarray(x[b].T.astype(BF16)),
                wq=chunked(W_q[:, base + idx_local]),
                wk=chunked(W_k[:, base + idx_local]),
                wv=chunked(W_v[:, base:base + DC]),
                wo=chunked2(W_o[base:base + DC, :]),
                cosr=cos_np,
                sinr=sin_np,
                cmask=cmask_np,
            )
        )
    return in_maps


def kernel(x, W_q, W_k, W_v, W_o, _trace=False, _trace_cores=None):
    from concourse.bass_utils import run_bass_kernel_spmd

    x = np.asarray(x, dtype=np.float32)
    W_q = np.asarray(W_q, dtype=np.float32)
    W_k = np.asarray(W_k, dtype=np.float32)
    W_v = np.asarray(W_v, dtype=np.float32)
    W_o = np.asarray(W_o, dtype=np.float32)

    nc = get_program()
    in_maps = make_in_maps(x, W_q, W_k, W_v, W_o)
    res = run_bass_kernel_spmd(
        nc,
        in_maps,
        list(range(NCORES)),
        trace=_trace,
        trace_cores=_trace_cores,
    )
    y = np.zeros((B, S, E), np.float32)
    for c in range(NCORES):
        y[c // 4] += res.results[c]["y"]
    if _trace:
        return y, res
    return y
